# revision 26
# baseline (speedup 1.0000x reference)
"""Trainium2 Bass kernel for nn_DiffusionLoss (B=4, N=2048).

Decomposition
-------------
loss = align_term + bond_term, pooled over the whole batch, then scaled by
the per-sample ht factor.

* align term + all O(N) reductions (means, 3x3 Kabsch matrix, SVD, rotated
  residual norms) are tiny -> host numpy in f64.
* bond term: sum_ij w_i w_j (dp_ij - dg_ij)^2
    = sum_ij w_i w_j dp^2 + sum_ij w_i w_j dg^2 - 2 * sum_ij w_i w_j dp dg.
  The two squared terms expand analytically to O(N) sums (host, f64, exact).
  Only the cross term P = sum_ij w_i w_j dp_ij dg_ij needs the full N x N
  pairwise pass -> device.

Device scheme
-------------
With augmented 5-vectors vp_i=[-2xp_i,1,|xp_i|^2], up_j=[xp_j,|xp_j|^2,1]
(and likewise for the ground truth), the 25-dim outer products satisfy
  (w_i^2 vp_i(x)vg_i) . (w_j^2 up_j(x)ug_j) = w_i^2 w_j^2 d2p_ij d2g_ij,
so one matmul per [128 rows x chunk] produces  v_ij + eps*w_i^2  directly,
where v = (w_i w_j dp dg)^2 >= 0.

The PE's f32r mode rounds matmul inputs to 11 explicit mantissa bits
(drop-12 round-to-nearest; verified bit-exact against the runtime
simulator), whose dot-product noise would swamp any usable eps.  Each of
the 25 features (+ the eps row) is therefore hi/lo SPLIT into f32r-exact
halves -- rows (hi,hi), (hi,lo), (lo,hi), K = 76 -- restoring fp32-level
products, so eps=48 strictly dominates the residual noise and every PSUM
value is positive.  No clamp pass, no masks.

Consumers: sqrt lives ONLY on the ACT engine on TRN2 (the DVE/Pool ISAs
reject pow/divide, and GPSIMD cannot read PSUM), so each 512-col PSUM bank
generation is consumed by activation(Sqrt) with the row-sum fused via
accum_out (in-place PSUM output, one [128,2048] span per instruction).
Because w_i is folded into the lhsT, every accumulator column is a plain
partial sum of w_i w_j dp dg terms -- the host just adds them all up.

Triangle at 256-col grain: block (local i, rows 128) covers cols
[256i, 2048): a 256-wide diagonal chunk (0.25-scaled lhsT copy Vq -- its
pairs appear in both orientations across the two cores, and sqrt of the
quarter gives 0.5x, restored by the host's uniform 2x) plus 256/512-wide
off-diagonal chunks.  256-wide matmuls are paired into one 512-col PSUM
bank generation sharing a start/stop accumulation group: 24 matmuls,
18 generations, 9216 columns per core (vs 10240 at 512-grain).

Pipeline: generations stream through the 8 PSUM banks (ring); ACT consumes
4-generation groups; the PE re-uses a bank only after the owning group's
semaphore.  Inputs arrive as five staggered DMA pieces across the Pool/SP/
ACT queues, ordered so each generation's operands land just in time; ACT
preloads the Sqrt activation table during the DMA wait and ships the
result tile at the end.

Sharding: core c -> batch c//2; parity c%2 takes row-blocks r with r%2==c%2.
"""

from contextlib import ExitStack

import numpy as np

import concourse.bass as bass
from concourse import mybir
from concourse.bass_utils import run_bass_kernel_spmd

B = 4
N = 2048
SIGMA_DATA = 16.0
EPS = 48.0

F32 = mybir.dt.float32

# 256-grain triangle.  Block (local i, global r = 2i + parity) covers cols
# [256*i, 2048): a 256-wide diagonal chunk (quarter-scaled lhsT) at
# [256i, 256i+256), then (for even i) a 256-wide remainder and 512-wide
# chunks.  Each PSUM bank generation ("gen") is 512 columns written by one
# 512-wide matmul or a pair of 256-wide matmuls sharing one start/stop
# accumulation group.  Entries: (block, ucol, width, use_vq).
def _d(i):
    return (i, 256 * i, 256, True)


def _rm(i):
    return (i, 256 * (i + 1), 256, False)


def _s(i, c):
    return (i, c, 512, False)


# Ordered so each gen's inputs arrive in DMA-piece order: piece A (V0,Vq0,
# U cols 0-512), B1 (U 512-1024), B2 (U 1024-2048), C1 (V1,V2), C2 (V3-7).
GENS = [
    [_d(0), _rm(0)],            # A
    [_s(0, 512)],               # B1
    [_d(1), _d(2)],             # C1 + B1 (d2 cols 512-768)
    [_s(1, 512)],               # B1
    [_rm(2), _d(3)],            # B1 (cols 768-1024)
    [_s(0, 1024)],              # B2 from here
    [_s(1, 1024)],
    [_s(0, 1536)],
    [_s(1, 1536)],
    [_s(2, 1024)],
    [_s(2, 1536)],
    [_s(3, 1024)],
    [_s(3, 1536)],
    [_d(4), _rm(4)],            # C2 from here
    [_s(4, 1536)],
    [_d(5), _d(6)],
    [_s(5, 1536)],
    [_rm(6), _d(7)],
]
assert len(GENS) == 18
assert sum(len(g) for g in GENS) == 24

# first matmul index of each gen, and total count
GEN_M0 = []
_m = 0
for _g in GENS:
    GEN_M0.append(_m)
    _m += len(_g)
N_MM = _m

# Consumer schedule (ACT only -- the DVE/Pool ISAs have no sqrt/pow and
# GPSIMD cannot access PSUM): groups of gens, each one contiguous span of
# one psum tensor.
GROUPS = [
    ("A", [0, 1, 2, 3]),
    ("A", [4, 5, 6, 7]),
    ("A", [8, 9, 10, 11]),
    ("A", [12, 13, 14, 15]),
    ("A", [16, 17]),
]
GROUP_OF_GEN = {}
for _gi, (_e, _chs) in enumerate(GROUPS):
    for _c in _chs:
        GROUP_OF_GEN[_c] = _gi
_ORD = {"A": 0, "D": 0}
GROUP_ORD = []
for _e, _ in GROUPS:
    _ORD[_e] += 1
    GROUP_ORD.append(_ORD[_e])
N_GROUPS = {"A": _ORD["A"], "D": _ORD["D"]}

# uv column layout:
# [V0|Vq0 (256) | U (2048) | V1|Vq1|V2|Vq2 (512) | V3..7|Vq3..7 (1280)]
UVW = 4096
U_OFF = 256

# The PE's f32r mode rounds inputs to 11 explicit mantissa bits (drop-12,
# round-to-nearest -- verified bit-exact against the runtime simulator).  A
# hi/lo split per feature restores fp32-level products: f = hi + lo with
# both parts f32r-exact, and  f*g ~ hi*g_hi + hi*g_lo + lo*g_hi  (the
# dropped lo*lo term is ~2^-24 relative).  3 K-rows per feature + the eps
# row -> K = 76.
KROWS = 76

_NC_CACHE = None


def _build_nc():
    nc = bass.Bass("TRN2", target_bir_lowering=False, debug=False, num_devices=8)

    F32R = mybir.dt.float32r
    uv = nc.declare_dram_parameter("uv", [KROWS, UVW], F32R, isOutput=False)
    res = nc.declare_dram_parameter("res", [128, len(GROUPS)], F32, isOutput=True)

    def vcol(i, quarter):
        if i == 0:
            base = 0
        elif i <= 2:
            base = 2304 + (i - 1) * 256
        else:
            base = 2816 + (i - 3) * 256
        return base + (128 if quarter else 0)

    with (
        nc.sbuf_tensor([KROWS, UVW], F32R) as uv_t,
        nc.sbuf_tensor([128, len(GROUPS)], F32) as res_t,
        nc.sbuf_tensor([128, 8], F32) as dummy,
        nc.psum_tensor([128, 2048], F32) as psum0,
        nc.psum_tensor([128, 2048], F32) as psum1,
        ExitStack() as stack,
        nc.Block() as block,
    ):
        sems = {
            name: stack.enter_context(nc.semaphore(name))
            for name in (
                "dma_a", "dma_b1", "dma_b2", "dma_c1", "dma_c2", "pe_sem",
                "act_sem", "dve_sem", "out_sem", "init_sem",
            )
        }
        (dma_a, dma_b1, dma_b2, dma_c1, dma_c2, pe_sem, act_sem, dve_sem,
         out_sem, init_sem) = (
            sems["dma_a"], sems["dma_b1"], sems["dma_b2"], sems["dma_c1"],
            sems["dma_c2"], sems["pe_sem"], sems["act_sem"], sems["dve_sem"],
            sems["out_sem"], sems["init_sem"],
        )
        psums = [psum0, psum1]

        def span(gi):
            # group gi covers a contiguous span within one psum tensor
            chs = GROUPS[gi][1]
            pt = psums[chs[0] % 8 // 4]
            c0 = (chs[0] % 4) * 512
            return pt[:, c0 : c0 + 512 * len(chs)]

        # which DMA piece a matmul operand needs
        def v_piece(i):
            return dma_a if i == 0 else (dma_c1 if i <= 2 else dma_c2)

        def u_piece(ucol):
            uvc = U_OFF + ucol
            if uvc < 768:
                return dma_a
            return dma_b1 if uvc < 1280 else dma_b2

        ENG_SEM = {"A": act_sem, "D": dve_sem}

        @block.sync
        def _(sync):
            # SP queue: U chunk 1 first (needed early), then U chunks 2-3
            sync.dma_start(
                out=uv_t[:, 768:1280], in_=uv[:, 768:1280]
            ).then_inc(dma_b1, 16)
            sync.dma_start(
                out=uv_t[:, 1280:2304], in_=uv[:, 1280:2304]
            ).then_inc(dma_b2, 16)
            sync.wait_ge(out_sem, 16)

        @block.gpsimd
        def _(gp):
            # priority piece: V0 | Vq0 | U0 -> first gen can start
            gp.dma_start(out=uv_t[:, :768], in_=uv[:, :768]).then_inc(dma_a, 16)

        @block.tensor
        def _(tensor):
            waited = set()
            last_ring = None

            def wait_piece(sem):
                if id(sem) not in waited:
                    tensor.wait_ge(sem, 16)
                    waited.add(id(sem))

            for g, gen in enumerate(GENS):
                if g >= 8:
                    grp = GROUP_OF_GEN[g - 8]
                    ring = (GROUPS[grp][0], GROUP_ORD[grp])
                    if ring != last_ring:
                        tensor.wait_ge(ENG_SEM[ring[0]], ring[1])
                        last_ring = ring
                pt = psums[g % 8 // 4]
                c0 = (g % 4) * 512
                off = 0
                for mi, (i, ucol, width, quarter) in enumerate(gen):
                    wait_piece(v_piece(i))
                    wait_piece(u_piece(ucol))
                    vc = vcol(i, quarter)
                    uvc = U_OFF + ucol
                    nc.tensor.matmul(
                        pt[:, c0 + off : c0 + off + width],
                        uv_t[:, vc : vc + 128],
                        uv_t[:, uvc : uvc + width],
                        start=(mi == 0),
                        stop=(mi == len(gen) - 1),
                    ).then_inc(pe_sem, 1)
                    off += width

        @block.vector
        def _(vector):
            for gi, (eng, chs) in enumerate(GROUPS):
                if eng != "D":
                    continue
                last_gen = chs[-1]
                vector.wait_ge(pe_sem, GEN_M0[last_gen] + len(GENS[last_gen]))
                nc.vector.tensor_scalar(
                    out=span(gi),
                    in0=span(gi),
                    scalar1=1.0,
                    scalar2=None,
                    op0=mybir.AluOpType.mult,
                    op1=mybir.AluOpType.add,
                    accum_out=res_t[:, gi : gi + 1],
                ).then_inc(dve_sem, 1)

        @block.scalar
        def _(scalar):
            # V blocks 1-2 then 3-7 on the ACT DMA queue
            scalar.dma_start(
                out=uv_t[:, 2304:2816], in_=uv[:, 2304:2816]
            ).then_inc(dma_c1, 16)
            scalar.dma_start(
                out=uv_t[:, 2816:], in_=uv[:, 2816:]
            ).then_inc(dma_c2, 16)
            # preload the Sqrt activation table during the DMA wait
            nc.scalar.memzero(dummy[:, :]).then_inc(init_sem, 1)
            scalar.wait_ge(init_sem, 1)
            nc.scalar.activation(
                out=dummy[:, :],
                in_=dummy[:, :],
                func=mybir.ActivationFunctionType.Sqrt,
            ).then_inc(init_sem, 1)
            for gi, (eng, chs) in enumerate(GROUPS):
                if eng != "A":
                    continue
                last_gen = chs[-1]
                scalar.wait_ge(pe_sem, GEN_M0[last_gen] + len(GENS[last_gen]))
                nc.scalar.activation(
                    out=span(gi),
                    in_=span(gi),
                    func=mybir.ActivationFunctionType.Sqrt,
                    accum_out=res_t[:, gi : gi + 1],
                ).then_inc(act_sem, 1)
            # drain own accum write, then wait for the other engine's columns
            scalar.wait_ge(act_sem, N_GROUPS["A"])
            if N_GROUPS["D"]:
                scalar.wait_ge(dve_sem, N_GROUPS["D"])
            scalar.dma_start(out=res[:, :], in_=res_t[:, :]).then_inc(
                out_sem, 16
            )

    return nc


def _augmented(xp32, xg32, w32):
    """Per-sample 26-feature tensors: U26 [B,N,26] (rhs, w_j^2-folded),
    V26 [B,N,26] (lhsT side, w_i^2-folded, eps row last)."""
    sp = np.sum(xp32 * xp32, axis=-1)
    sg = np.sum(xg32 * xg32, axis=-1)
    ones = np.ones((B, N, 1), np.float32)
    up = np.concatenate([xp32, sp[..., None], ones], axis=-1)
    up = up * (w32**2)[..., None]
    ug = np.concatenate([xg32, sg[..., None], ones], axis=-1)
    vp = np.concatenate([-2.0 * xp32, ones, sp[..., None]], axis=-1)
    vg = np.concatenate([-2.0 * xg32, ones, sg[..., None]], axis=-1)

    U = np.einsum("nja,njc->njac", up, ug).reshape(B, N, 25).astype(np.float32)
    V = np.einsum("nia,nic->niac", vp, vg).reshape(B, N, 25).astype(np.float32)
    V = V * (w32**2)[..., None]
    U26 = np.concatenate([U, ones], axis=-1)
    V26 = np.concatenate(
        [V, (EPS * w32**2)[..., None].astype(np.float32)], axis=-1
    )
    return U26, V26


def _q12(x):
    """Round-to-nearest f32r quantization (drop 12 mantissa bits) --
    bit-exact match of the PE's f32r input rounding."""
    xi = np.ascontiguousarray(x, dtype=np.float32).view(np.int32)
    return ((xi + 0x800) & ~0xFFF).view(np.float32)


def _split76(U26, V26):
    """[B,N,26] f32 feature pairs -> f32r-exact hi/lo split [B,76,N]."""
    Uh = _q12(U26)
    Ul = _q12(U26 - Uh)
    Vh = _q12(V26)
    Vl = _q12(V26 - Vh)
    U76 = np.empty((B, KROWS, N), np.float32)
    V76 = np.empty((B, KROWS, N), np.float32)
    for k in range(25):
        U76[:, 3 * k] = Uh[..., k].transpose(0, 1)
        U76[:, 3 * k + 1] = Ul[..., k]
        U76[:, 3 * k + 2] = Uh[..., k]
        V76[:, 3 * k] = Vh[..., k]
        V76[:, 3 * k + 1] = Vh[..., k]
        V76[:, 3 * k + 2] = Vl[..., k]
    U76[:, 75] = Uh[..., 25]  # 1.0
    V76[:, 75] = Vh[..., 25]  # eps * w_i^2 (f32r-rounded; only a bias)
    return U76, V76


def _host_inputs(U26, V26):
    U76, V76 = _split76(U26, V26)
    in_maps = []
    for core in range(8):
        b, h = core // 2, core % 2
        blocks = [2 * i + h for i in range(8)]
        Vb = [V76[b, :, r * 128 : (r + 1) * 128] for r in blocks]
        Vqb = [0.25 * v for v in Vb]
        # [V0|Vq0 | U | V1|Vq1|V2|Vq2 | V3..7|Vq3..7]
        pieces = [Vb[0], Vqb[0], U76[b]]
        for i in range(1, 8):
            pieces += [Vb[i], Vqb[i]]
        cols = np.concatenate(pieces, axis=1)
        in_maps.append({"uv": np.ascontiguousarray(cols)})
    return in_maps


def _host_assemble(xp32, xg32, ht32, w32, P):
    """Alignment loss + analytic bond parts + final scaling (f64)."""
    xp = xp32.astype(np.float64)
    xg = xg32.astype(np.float64)
    ht = ht32.astype(np.float64)
    w = w32.astype(np.float64)

    W = w.sum(axis=1)  # [B]
    # weighted_rigid_align(x_l=xGT, xGT_l=xpred, w): align GT onto pred frame
    mu = (w[..., None] * xg).sum(axis=1) / W[:, None]
    muGT = (w[..., None] * xp).sum(axis=1) / W[:, None]
    xc = xg - mu[:, None, :]
    xGTc = xp - muGT[:, None, :]
    M = np.einsum("bni,bnj->bij", w[..., None] * xGTc, xc)
    U, _, Vh = np.linalg.svd(M)
    R = U @ Vh
    det = np.linalg.det(R)
    Fm = np.diag([1.0, 1.0, -1.0])
    Rfix = np.einsum("bij,jk,bkl->bil", U, Fm, Vh)
    R = np.where(det[:, None, None] < 0, Rfix, R)
    xalign = np.einsum("bnj,bkj->bnk", xc, R) + muGT[:, None, :]
    lnum = (np.linalg.norm(xp - xalign, axis=-1) * w).sum()
    loss_align = lnum / W.sum()

    sp = (xp * xp).sum(-1)
    sg = (xg * xg).sum(-1)
    wxp = np.einsum("bn,bni->bi", w, xp)
    wxg = np.einsum("bn,bni->bi", w, xg)
    Ap = 2 * (W * (w * sp).sum(1) - (wxp**2).sum(1))
    Bg = 2 * (W * (w * sg).sum(1) - (wxg**2).sum(1))

    bond = (Ap + Bg - 2 * P).sum() / (W**2).sum()
    loss = loss_align + bond
    out = (ht**2 + SIGMA_DATA**2) / (ht + SIGMA_DATA) ** 2 * loss
    return out.astype(np.float32)


def kernel(xpred_l, xGT_l, ht, w_l):
    global _NC_CACHE
    xp32 = np.ascontiguousarray(np.asarray(xpred_l, dtype=np.float32))
    xg32 = np.ascontiguousarray(np.asarray(xGT_l, dtype=np.float32))
    ht32 = np.asarray(ht, dtype=np.float32)
    w32 = np.ascontiguousarray(np.asarray(w_l, dtype=np.float32))

    if _NC_CACHE is None:
        _NC_CACHE = _build_nc()
    nc = _NC_CACHE

    U26, V26 = _augmented(xp32, xg32, w32)
    in_maps = _host_inputs(U26, V26)
    results = run_bass_kernel_spmd(nc, in_maps, list(range(8))).results

    # every res column is a partial sum of 0.5^[diag] * sqrt(v + eps w_i^2);
    # double for ordered-pair counting, subtract the (i,i) eps artifacts.
    P = np.zeros(B)
    for core in range(8):
        b = core // 2
        P[b] += 2.0 * results[core]["res"].astype(np.float64).sum()
    P -= np.sqrt(EPS) * w32.astype(np.float64).sum(axis=1)

    return _host_assemble(xp32, xg32, ht32, w32, P)
